# revision 48
# baseline (speedup 1.0000x reference)
"""Llama GQA attention (B=2, T=2048, C=2048, 32 Q heads / 8 KV heads, D=64,
interleaved RoPE, causal) on 8 TRN2 NeuronCores.

Sharding: core c -> (batch b = c//4, head-group g = c%4). Each core handles one
batch element and 8 Q heads / 2 KV heads (wq/wk/wv column shards, wo row
shard), producing a partial y[b]; the host sums the 4 partials per batch.

v2: all-bf16 matmul operands (fp32 PSUM accumulation), xbar DMA-transpose for
x^T, all-t-block-resident intermediates for scheduling freedom, row-packed QK
head pairs, causal width restriction on diagonal tiles, batched reciprocal,
masks on Pool / exp-only ACT.
"""
import sys

sys.path.insert(0, "/opt/trn_rl_repo")
sys.path.insert(0, "/opt/trn_rl_repo/concourse")

import numpy as np

import concourse.bass as bass
import concourse.tile as tile
from concourse import bacc, mybir
from concourse.bass_utils import run_bass_kernel_spmd
from concourse.masks import make_identity

T = 2048
C = 2048
D = 64
HQ = 8          # q heads per core
HKV = 2         # kv heads per core
QCOLS = HQ * D  # 512
KCOLS = HKV * D  # 128
TB = 512        # t-block (i-block) size
NTB = T // TB   # 4
CCH = C // 128  # 16 c-chunks
SCALE = 0.125   # 1/sqrt(64)

f32 = mybir.dt.float32
bf16 = mybir.dt.bfloat16
EXPF = mybir.ActivationFunctionType.Exp

_cache = {}


def _build_program():
    nc = bacc.Bacc("TRN2", target_bir_lowering=False, debug=False, num_devices=1)
    d = {}
    d["x"] = nc.dram_tensor("x", [T, C], bf16, kind="ExternalInput").ap()
    d["wq"] = nc.dram_tensor("wq", [C, QCOLS], bf16, kind="ExternalInput").ap()
    d["wk"] = nc.dram_tensor("wk", [C, KCOLS], bf16, kind="ExternalInput").ap()
    d["wv"] = nc.dram_tensor("wv", [C, KCOLS], bf16, kind="ExternalInput").ap()
    d["wo"] = nc.dram_tensor("wo", [QCOLS, C], bf16, kind="ExternalInput").ap()
    d["cos"] = nc.dram_tensor("cos", [128, T], bf16, kind="ExternalInput").ap()
    d["sin"] = nc.dram_tensor("sin", [128, T], bf16, kind="ExternalInput").ap()
    d["msk"] = nc.dram_tensor("msk", [128, 4, TB], bf16, kind="ExternalInput").ap()
    y_d = nc.dram_tensor("y", [T, C], f32, kind="ExternalOutput").ap()

    with tile.TileContext(nc) as tc:
        _emit(nc, tc, d, y_d)
    nc.compile()
    return nc


def _emit(nc, tc, d, y_d):
    from contextlib import ExitStack

    ctx = ExitStack()
    with ctx:
        sb_w = ctx.enter_context(tc.tile_pool(name="sb_w", bufs=1))
        sb = ctx.enter_context(tc.tile_pool(name="sb", bufs=1))
        sb_t = ctx.enter_context(tc.tile_pool(name="sb_t", bufs=2))
        sb_p = ctx.enter_context(tc.tile_pool(name="sb_p", bufs=4))
        sb_ou = ctx.enter_context(tc.tile_pool(name="sb_ou", bufs=5))
        sb_y = ctx.enter_context(tc.tile_pool(name="sb_y", bufs=1))
        ps_qk = ctx.enter_context(tc.tile_pool(name="ps_qk", bufs=2, space="PSUM"))
        ps_av = ctx.enter_context(tc.tile_pool(name="ps_av", bufs=2, space="PSUM"))
        ps_mm = ctx.enter_context(tc.tile_pool(name="ps_mm", bufs=2, space="PSUM"))
        ps_y = ctx.enter_context(tc.tile_pool(name="ps_y", bufs=2, space="PSUM"))

        # ---- weights (DMAs issued below, interleaved with x transposes) ----
        wq_sb = sb_w.tile([128, CCH, QCOLS], bf16)
        wk_sb = sb_w.tile([128, CCH, KCOLS], bf16)
        wv_sb = sb_w.tile([128, CCH, KCOLS], bf16)
        wo_sb = sb_w.tile([128, 4, C], bf16)

        # ---- constants ----
        ident_f = sb.tile([128, 128], f32)
        make_identity(nc, ident_f[:])
        ident = sb.tile([128, 128], bf16)
        nc.vector.tensor_copy(ident[:], ident_f[:])
        cs = sb.tile([128, T], bf16)
        sn = sb.tile([128, T], bf16)
        mskt = sb.tile([128, 4, TB], bf16)
        masks = [mskt[:, mi, :] for mi in range(4)]

        # ---- persistent state (all t-blocks resident) ----
        xt = [sb.tile([128, CCH, TB], bf16, tag=f"xt{i}", name=f"xt{i}")
              for i in range(NTB)]
        qt = [[sb.tile([128, TB], bf16, tag=f"qt{i}s{s}", name=f"qt{i}s{s}")
               for s in range(4)] for i in range(NTB)]
        kt = [sb.tile([128, TB], bf16, tag=f"kt{i}", name=f"kt{i}")
              for i in range(NTB)]
        # v tiles: [v_kv0 (64) | ones | v_kv1 (64) | ones]
        v_tiles = [sb.tile([128, 2 * (D + 1)], bf16, tag=f"v{i}", name=f"v{i}")
                   for i in range(NTB * 4)]
        for vt in v_tiles:
            nc.gpsimd.memset(vt[:, D:D + 1], 1.0)
            nc.gpsimd.memset(vt[:, 2 * D + 1:2 * D + 2], 1.0)
        ot = [[sb.tile([128, TB], bf16, tag=f"ot{i}o{o}", name=f"ot{i}o{o}")
               for o in range(4)] for i in range(NTB)]

        # ---- input DMAs.  Weight/const loads are single instructions with
        # per-partition-contiguous DRAM reads (128 descriptors each) on the
        # SWDGE path; x^T is one xbar-transpose instruction per t-block on
        # HWDGE.  Channel c = p*16+cc lands at slot (p, cc) on BOTH the
        # transpose and the "(p c) n -> p c n" weight loads, so the
        # projection contraction stays aligned with no host permutation
        # (wo is host-shuffled to match its oc-major consumer layout).
        nc.gpsimd.dma_start(wq_sb[:], d["wq"].rearrange("(p c) n -> p c n",
                                                        c=CCH))
        xin0 = []
        for tp in range(4):
            xin = sb_t.tile([128, C], bf16, tag="xin", bufs=2,
                            name=f"xin{tp}")
            nc.sync.dma_start(xin[:], d["x"][tp * 128:(tp + 1) * 128, :])
            xin0.append(xin)
        nc.sync.dma_start(xt[1][:], d["x"][TB:2 * TB, :], transpose=True)
        nc.gpsimd.dma_start(wk_sb[:], d["wk"].rearrange("(p c) n -> p c n",
                                                        c=CCH))
        nc.gpsimd.dma_start(wv_sb[:], d["wv"].rearrange("(p c) n -> p c n",
                                                        c=CCH))
        nc.gpsimd.dma_start(cs[:], d["cos"][:, :])
        nc.gpsimd.dma_start(sn[:], d["sin"][:, :])
        nc.gpsimd.dma_start(mskt[:], d["msk"][:, :, :])
        for tb in range(2, NTB):
            nc.sync.dma_start(xt[tb][:], d["x"][tb * TB:(tb + 1) * TB, :],
                              transpose=True)
        nc.gpsimd.dma_start(wo_sb[:], d["wo"].rearrange("(p o) n -> p o n",
                                                        o=4))

        def rope_tile(psum, dests):
            """psum [128,512]: per 64-row head-block [E(32)|O(32)] layout.
            dests: list of (dst_tile, dst_base, idx) — one per 64-row block
            covering psum rows idx*64..idx*64+64.  snAlt holds -sin on O-rows
            so both combining adds keep equal SBUF base partitions."""
            i0 = dests[0][3]
            pqs = sb_t.tile([128, TB], bf16, tag="ropepq")
            nc.scalar.copy(pqs[:], psum[:])
            c1 = sb_t.tile([128, TB], bf16, tag="ropec1")
            c2 = sb_t.tile([128, TB], bf16, tag="ropec2")
            nc.vector.tensor_mul(c1[:], pqs[:], cs[:, i0:i0 + TB])
            for b in range(4):  # 32-row blocks; write E<->O swapped
                sw = b ^ 1
                nc.vector.tensor_mul(c2[sw * 32:sw * 32 + 32, :],
                                     pqs[b * 32:b * 32 + 32, :],
                                     sn[b * 32:b * 32 + 32, i0:i0 + TB])
            for dst, base, idx, _ in dests:
                r = idx * 64
                nc.vector.tensor_add(dst[base:base + 64, :],
                                     c1[r:r + 64, :], c2[r:r + 64, :])

        # ---- phase B generator: yields every ~2 matmuls so projection
        # work can be woven into the attention stream as PE filler ----
        def b_units(tb):
            i0 = tb * TB
            if tb == 0:
                # x^T for block 0 on the PE (beats the xbar path's startup
                # latency and doubles as HAM warm-up); 4 transposes fill one
                # [128,512] psum slot, one copy drains it into 4 cc-chunks.
                for tp in range(4):
                    for cg in range(4):
                        px = ps_mm.tile([128, TB], bf16, tag="pm", name="px")
                        for c2 in range(4):
                            cc = cg * 4 + c2
                            nc.tensor.transpose(
                                px[:, c2 * 128:(c2 + 1) * 128],
                                xin0[tp][:, cc * 128:(cc + 1) * 128], ident[:])
                        nc.vector.tensor_copy(
                            xt[0][:, cg * 4:(cg + 1) * 4,
                                  tp * 128:(tp + 1) * 128],
                            px[:].rearrange("p (c t) -> p c t", c=4))
                        yield
            pk = ps_mm.tile([128, TB], f32, tag="pm")
            for cc in range(CCH):
                nc.tensor.matmul(pk[:], wk_sb[:, cc, :], xt[tb][:, cc, :],
                                 start=(cc == 0), stop=(cc == CCH - 1))
                if cc % 2 == 1:
                    yield
            rope_tile(pk, [(kt[tb], 0, 0, i0), (kt[tb], 64, 1, i0)])
            yield

            # Q^T in 128-row chunks; head g_h = 2j+h2 -> partition half
            # g_h//4, slot g_h%4 (matches kt halves for the paired QK).
            # j order 0,2,1,3 completes pair-slots 0/1 first.
            for j in (0, 2, 1, 3):
                pq = ps_mm.tile([128, TB], f32, tag="pm")
                for cc in range(CCH):
                    nc.tensor.matmul(pq[:], wq_sb[:, cc, j * 128:(j + 1) * 128],
                                     xt[tb][:, cc, :], start=(cc == 0),
                                     stop=(cc == CCH - 1))
                    if cc % 2 == 1:
                        yield
                dests = []
                for h2 in range(2):
                    g_h = 2 * j + h2
                    dests.append((qt[tb][g_h % 4][:, :], (g_h // 4) * 64,
                                  h2, i0))
                rope_tile(pq, dests)
                yield

            pv = ps_mm.tile([128, TB], f32, tag="pm")
            for cc in range(CCH):
                nc.tensor.matmul(pv[:], wv_sb[:, cc, :], xt[tb][:, cc, :],
                                 start=(cc == 0), stop=(cc == CCH - 1))
                if cc % 2 == 1:
                    yield
            vt_tmp = sb_t.tile([128, TB], bf16, tag="vt_tmp")
            nc.vector.tensor_copy(vt_tmp[:], pv[:])
            for j2 in range(4):
                pvt = ps_mm.tile([128, KCOLS], bf16, tag="pm")
                nc.tensor.transpose(pvt[:, 0:KCOLS],
                                    vt_tmp[:, j2 * 128:(j2 + 1) * 128], ident[:])
                vt = v_tiles[tb * 4 + j2]
                for kv in range(2):
                    nc.vector.tensor_copy(vt[:, kv * (D + 1):kv * (D + 1) + D],
                                          pvt[:, kv * D:(kv + 1) * D])
                yield

        # ---- phase C (attention) and D (o_proj) emitters ----
        def phase_c(tb, take, rate):
            njt = 4 * (tb + 1)
            dd = sb_t.tile([97, 2, TB], f32, tag="dd", bufs=1)
            ous = []
            for slot in range(4):  # head pair (slot, slot+4)
                po_a = ps_av.tile([128, TB], f32, tag="po")
                po_b = ps_av.tile([128, TB], f32, tag="po")
                for jt in range(njt):
                    ktile = kt[jt // 4]
                    jsl = slice((jt % 4) * 128, (jt % 4) * 128 + 128)
                    di = jt - 4 * tb  # diagonal index (>=0 on diagonal)
                    off = di * 128 if di > 0 else 0
                    pa = ps_qk.tile([128, TB], f32, tag="ps")
                    pb = ps_qk.tile([128, TB], f32, tag="ps")
                    nc.tensor.matmul(pa[:, off:TB], ktile[0:64, jsl],
                                     qt[tb][slot][0:64, off:TB],
                                     start=True, stop=True)
                    nc.tensor.matmul(pb[:, off:TB], ktile[64:128, jsl],
                                     qt[tb][slot][64:128, off:TB],
                                     start=True, stop=True)
                    p_a = sb_p.tile([128, TB], bf16, tag="p_sb")
                    p_b = sb_p.tile([128, TB], bf16, tag="p_sb")
                    take(rate)
                    nc.scalar.activation(p_a[:, off:TB], pa[:, off:TB], EXPF,
                                         bias=0.0, scale=SCALE)
                    nc.scalar.activation(p_b[:, off:TB], pb[:, off:TB], EXPF,
                                         bias=0.0, scale=SCALE)
                    if di >= 0:
                        msl = masks[di][:, off:TB]
                        nc.gpsimd.tensor_mul(p_a[:, off:TB], p_a[:, off:TB], msl)
                        nc.gpsimd.tensor_mul(p_b[:, off:TB], p_b[:, off:TB], msl)
                    nc.tensor.matmul(po_a[0:D + 1, off:TB],
                                     v_tiles[jt][:, 0:D + 1], p_a[:, off:TB],
                                     start=(jt == 0), stop=(jt == njt - 1))
                    nc.tensor.matmul(po_b[0:D + 1, off:TB],
                                     v_tiles[jt][:, D + 1:2 * (D + 1)],
                                     p_b[:, off:TB],
                                     start=(jt == 0), stop=(jt == njt - 1))
                # drain PSUM fast (frees po banks); denominators gathered
                # at 32-aligned partitions of the per-tb dd tile
                ou = sb_ou.tile([65, 2, TB], bf16, tag="ou")
                nc.vector.tensor_copy(ou[:, 0, :], po_a[0:D + 1, :])
                nc.vector.tensor_copy(ou[:, 1, :], po_b[0:D + 1, :])
                nc.vector.tensor_copy(dd[slot * 32:slot * 32 + 1, 0, :],
                                      ou[D:D + 1, 0, :])
                nc.vector.tensor_copy(dd[slot * 32:slot * 32 + 1, 1, :],
                                      ou[D:D + 1, 1, :])
                ous.append(ou)
            # normalize tail — overlaps D(tb-1) o_proj on the PE
            rr = sb_t.tile([97, 2, TB], f32, tag="rr", bufs=1)
            nc.vector.reciprocal_approx_fast(rr[:, 0, :], dd[:, 0, :])
            nc.vector.reciprocal_approx_fast(rr[:, 1, :], dd[:, 1, :])
            for h in range(HQ):
                rtmp = sb_t.tile([1, TB], bf16, tag="rtmp")
                nc.vector.tensor_copy(
                    rtmp[:], rr[(h % 4) * 32:(h % 4) * 32 + 1, h // 4, :])
                rb = sb_t.tile([64, TB], bf16, tag="rb")
                nc.gpsimd.partition_broadcast(rb[:], rtmp[:])
                nc.vector.tensor_mul(
                    ot[tb][h // 2][(h % 2) * 64:(h % 2) * 64 + 64, :],
                    ous[h % 4][0:D, h // 4, :], rb[:])

        def d_units(tb):
            i0 = tb * TB
            for t2 in range(4):
                y_sb = sb_y.tile([128, C], f32, tag="y_sb")
                for cb in range(4):
                    py = ps_y.tile([128, 512], f32, tag="py")
                    for oc in range(4):
                        nc.tensor.matmul(
                            py[:], ot[tb][oc][:, t2 * 128:(t2 + 1) * 128],
                            wo_sb[:, oc, cb * 512:(cb + 1) * 512],
                            start=(oc == 0), stop=(oc == 3))
                        if oc % 2 == 1:
                            yield
                    nc.vector.tensor_copy(y_sb[:, cb * 512:(cb + 1) * 512],
                                          py[:])
                nc.sync.dma_start(
                    y_d[i0 + t2 * 128:i0 + (t2 + 1) * 128, :], y_sb[:])
                yield

        # Emission: B(0) dense upfront; B(tb+1)/D(tb-1) woven into C(tb)'s
        # attention stream as PE filler.  CRITICAL: b_units(tb) must be fully
        # EMITTED before phase_c(tb) — Tile's dependency tracker follows
        # emission order, so a read emitted before its producer gets no
        # semaphore (reads stale SBUF).
        gens = []

        def take(k):
            while k > 0 and gens:
                if next(gens[0], "END") == "END":
                    gens.pop(0)
                else:
                    k -= 1

        def drain_through(g):
            if g not in gens:
                return
            while gens:
                g0 = gens.pop(0)
                for _ in g0:
                    pass
                if g0 is g:
                    break

        b = [b_units(t) for t in range(NTB)]
        for _ in b[0]:
            pass
        gens.append(b[1])
        phase_c(0, take, 2)
        drain_through(b[1])
        gens.append(b[2])
        gens.append(d_units(0))
        phase_c(1, take, 3)
        drain_through(b[2])
        gens.append(b[3])
        gens.append(d_units(1))
        phase_c(2, take, 3)
        drain_through(b[3])
        gens.append(d_units(2))
        phase_c(3, take, 2)
        while gens:
            drain_through(gens[-1])
        for _ in d_units(NTB - 1):
            pass


def _perm_cols(w):
    """Reorder each 64-wide head block's columns to [evens, odds]."""
    cols = []
    for h0 in range(0, w.shape[1], D):
        cols.extend(range(h0, h0 + D, 2))
        cols.extend(range(h0 + 1, h0 + D, 2))
    return np.ascontiguousarray(w[:, cols])


def _perm_rows(w):
    """Host shuffle so the contiguous "(p c) n -> p c n" DMA delivers row
    cc*128+p at slot (p, cc), matching the xbar transpose's channel order."""
    n = w.shape[1]
    return np.ascontiguousarray(
        w.reshape(CCH, 128, n).transpose(1, 0, 2).reshape(C, n))


def _host_tables():
    import ml_dtypes
    inv = (1.0 / (10000.0 ** (np.arange(0, D, 2) / D)))
    ang = np.arange(T)[None, :] * inv[:, None]          # [32, T]
    cos = np.tile(np.cos(ang), (4, 1)).astype(ml_dtypes.bfloat16)  # [128, T]
    # snAlt: +sin on E-input rows (blocks 0,2), -sin on O-input rows (1,3)
    s1 = np.sin(ang)
    sin = np.concatenate([s1, -s1, s1, -s1], axis=0).astype(ml_dtypes.bfloat16)
    msk = np.zeros((4, 128, TB), dtype=np.float32)
    for mi, off in enumerate((0, -128, -256, -384)):
        p = np.arange(128)[:, None]
        f = np.arange(TB)[None, :]
        msk[mi] = (off + f - p >= 0).astype(np.float32)
    msk = np.ascontiguousarray(msk.transpose(1, 0, 2))  # [128, 4, TB]
    return cos, sin, msk.astype(ml_dtypes.bfloat16)


def kernel(x, wq, wk, wv, wo, _trace=False):
    import ml_dtypes
    if "nc" not in _cache:
        _cache["nc"] = _build_program()
    nc = _cache["nc"]

    bf = ml_dtypes.bfloat16
    cos, sin, msk = _host_tables()
    in_maps = []
    for c in range(8):
        b, g = c // 4, c % 4
        in_maps.append({
            "x": np.ascontiguousarray(x[b]).astype(bf),
            "wq": _perm_rows(_perm_cols(
                wq[:, g * QCOLS:(g + 1) * QCOLS])).astype(bf),
            "wk": _perm_rows(_perm_cols(
                wk[:, g * KCOLS:(g + 1) * KCOLS])).astype(bf),
            "wv": _perm_rows(np.ascontiguousarray(
                wv[:, g * KCOLS:(g + 1) * KCOLS])).astype(bf),
            "wo": np.ascontiguousarray(
                wo[g * QCOLS:(g + 1) * QCOLS, :].reshape(4, 128, C)
                .transpose(1, 0, 2).reshape(QCOLS, C)).astype(bf),
            "cos": cos, "sin": sin, "msk": msk,
        })

    res = run_bass_kernel_spmd(nc, in_maps, core_ids=list(range(8)),
                               trace=_trace)
    _cache["last_res"] = res
    y = np.zeros((2, T, C), dtype=np.float32)
    for c in range(8):
        y[c // 4] += np.asarray(res.results[c]["y"], dtype=np.float32)
    return y
